# revision 1
# baseline (speedup 1.0000x reference)
"""Trainium2 Bass kernel for the CCA module (attention + 1x1 convs + BN/ReLU).

Contract: kernel(**inputs) takes the FULL fp32 inputs (shapes hardcoded below),
shards the batch over 8 NeuronCores (2 samples each), runs a Bass/Tile kernel
via run_bass_kernel_spmd, and returns the FULL (16, 512, 64, 64) fp32 output.

Host-side preprocessing (numpy):
  - BN (eval mode) folded into the 1x1 conv weights/biases.
  - Weights pre-transposed/chunked into the exact SBUF layouts the kernel uses.
  - Activations cast to bf16 (tensor engine runs bf16 @ 1 cycle/row; accumulate
    is always fp32 in PSUM).

Device-side per sample s (C=512, C8=64, HW=4096 pixels):
  projT[n,k] = sum_c x[c,n] * key_w[k,c]      (x slices as stationary operand)
  energy[k,q] = sum_n projT[n,k] * attT[n,q]  (attT via tensor-engine transpose)
  attn = softmax_q(energy)                    (max/exp/sum on ACT+DVE)
  W1a = W1' @ attn                            (c1 folded into a 64x64 matmul)
  out2[k,n] = relu(sum_q W1a[k,q] att[q,n] + b1[k])
  y[o,n] = relu(sum_c W2b[o,c] x[c,n] + sum_k W2a[o,k] out2[k,n] + b2[o])
"""

from contextlib import ExitStack

import numpy as np

import concourse.bacc as bacc
import concourse.tile as tile
from concourse import mybir
from concourse.bass_utils import run_bass_kernel_spmd
from concourse.masks import make_identity

N_CORES = 8
B, C, H, W = 16, 512, 64, 64
C8 = C // 8          # 64
HW = H * W           # 4096
S = B // N_CORES     # samples per core = 2
NCH = C // 128       # channel chunks = 4
NT = HW // 128       # 128-wide pixel tiles = 32
NT8 = HW // 512      # 512-wide pixel tiles = 8
EPS = 1e-5

BF16 = mybir.dt.bfloat16
F32 = mybir.dt.float32
NP_BF16 = mybir.dt.np(BF16)

_BUILT = None
PHASE_MARKS = []  # (label, n_insts_at_mark) for trace attribution


def _mark(nc, label):
    PHASE_MARKS.append((label, len(nc.inst_map)))


def _emit_loads(nc, pools, dram, s, x_first):
    (consts, xpool, attpool, attTpool, projTpool, out2pool, ypool, small,
     psA, psB, psC) = pools
    x_d, att_d = dram["x"], dram["att"]
    _mark(nc, f"loads_{s}")

    att_sb = attpool.tile([C8, HW], BF16, name=f"att_sb_{s}", tag="att")
    nc.sync.dma_start(out=att_sb, in_=att_d[s])
    # attT via DMA transpose: attT[p, nt, q] = att[q, nt*128 + p]
    attT = attTpool.tile([128, NT, C8], BF16, name=f"attT_{s}", tag="attT")
    nc.sync.dma_start_transpose(attT, att_d[s])
    # x in 4 n-quarters so compute can start when the first arrives
    x_sb = xpool.tile([128, NCH, HW], BF16, name=f"x_sb_{s}", tag="x")
    for q in range(4):
        nq = HW // 4
        nc.sync.dma_start(out=x_sb[:, :, q * nq:(q + 1) * nq],
                          in_=x_d[s][:, :, q * nq:(q + 1) * nq])
    return x_sb, att_sb, attT


def _emit_sample(nc, tc, pools, dram, s, loaded, next_loads):
    (consts, xpool, attpool, attTpool, projTpool, out2pool, ypool, small,
     psA, psB, psC) = pools
    y_d = dram["y"]
    kwT, kb_bc, w1T, b1, w2aT, w2bT, b2, ident, expbias = (
        dram["kwT_sb"], dram["kb_bc_sb"], dram["w1T_sb"], dram["b1_sb"],
        dram["w2aT_sb"], dram["w2bT_sb"], dram["b2_sb"], dram["ident"],
        dram["expbias"])
    x_sb, att_sb, attT = loaded

    _mark(nc, f"projT_{s}")
    # ---- projT = x^T @ key_w^T (+ key_b broadcast along free dim) ----------
    # 8 n-tiles share one PSUM bank (8 x 64 fp32 = 2KB), one evacuation each.
    projT = []
    for c in range(NT // 8):
        ps_p = psA.tile([128, 8, C8], F32, name=f"ps_p_{s}_{c}", tag="pt")
        for i in range(8):
            nt = c * 8 + i
            for ci in range(NCH):
                nc.tensor.matmul(
                    ps_p[:, i, :],
                    lhsT=x_sb[:, ci, nt * 128:(nt + 1) * 128],
                    rhs=kwT[:, ci, :],
                    start=(ci == 0), stop=(ci == NCH - 1))
        pj = projTpool.tile([128, 8, C8], BF16, name=f"pj_{s}_{c}", tag="pj",
                            bufs=8)
        nc.vector.tensor_add(pj, ps_p, kb_bc)
        projT.append(pj)

    # prefetch next sample's inputs (att first: its transpose gates energy)
    if next_loads is not None:
        next_loads()

    _mark(nc, f"energy_{s}")
    # ---- energy + softmax --------------------------------------------------
    ps_e = psB.tile([C8, C8], F32, name=f"ps_e_{s}", tag="sm")
    for nt in range(NT):
        nc.tensor.matmul(ps_e, lhsT=projT[nt // 8][:, nt % 8, :],
                         rhs=attT[:, nt, :],
                         start=(nt == 0), stop=(nt == NT - 1))
    def emit_c2x(ot, half):
        # the x-only part of a c2 accumulation group (no dependency on the
        # attention chain -> can fill PE bubbles during softmax)
        js = [half * 4 + i for i in range(4)]
        pts = [psC.tile([128, 512], F32, name=f"ps_y_{s}_{ot}_{j}", tag="c2")
               for j in js]
        for ci in range(NCH):
            for i, j in enumerate(js):
                nc.tensor.matmul(
                    pts[i],
                    lhsT=w2bT[:, ci, ot * 128:(ot + 1) * 128],
                    rhs=x_sb[:, ci, j * 512:(j + 1) * 512],
                    start=(ci == 0), stop=False)
        return pts

    hoisted = {}

    negmax = small.tile([C8, 1], F32, name=f"negmax_{s}")
    nc.vector.tensor_reduce(negmax, ps_e, axis=mybir.AxisListType.X,
                            op=mybir.AluOpType.max, negate=True)
    attn_exp = small.tile([C8, C8], F32, name=f"attn_exp_{s}")
    sumexp = small.tile([C8, 1], F32, name=f"sumexp_{s}")
    nc.scalar.activation(attn_exp, ps_e, mybir.ActivationFunctionType.Exp,
                         bias=negmax, scale=1.0, accum_out=sumexp)
    rec = small.tile([C8, 1], F32, name=f"rec_{s}")
    nc.vector.reciprocal(rec, sumexp)
    attn_bf = small.tile([C8, C8], BF16, name=f"attn_bf_{s}")
    nc.vector.tensor_scalar_mul(attn_bf, attn_exp, rec)

    # ---- W1aT = attn^T @ W1'^T = (W1' @ attn)^T in ONE matmul --------------
    # (c1 folded into the attention matrix; no transpose needed)
    ps_w = psB.tile([C8, C8], F32, name=f"ps_w_{s}", tag="sm")
    nc.tensor.matmul(ps_w, lhsT=attn_bf, rhs=w1T, start=True, stop=True)
    w1aT = small.tile([C8, C8], BF16, name=f"w1aT_{s}")
    nc.scalar.copy(w1aT, ps_w)

    _mark(nc, f"out2_{s}")
    # ---- out2 = relu(W1a @ att + b1) ---------------------------------------
    out2 = out2pool.tile([C8, HW], BF16, name=f"out2_{s}", tag="out2")
    for j in range(NT8):
        ps_o = psB.tile([C8, 512], F32, name=f"ps_o_{s}_{j}", tag="sm")
        nc.tensor.matmul(ps_o, lhsT=w1aT, rhs=att_sb[:, j * 512:(j + 1) * 512],
                         start=True, stop=True)
        nc.scalar.activation(out2[:, j * 512:(j + 1) * 512], ps_o,
                             mybir.ActivationFunctionType.Relu,
                             bias=b1, scale=1.0)

    _mark(nc, f"c2_{s}")
    # ---- c2: y = relu(W2b @ x + W2a @ out2 + b2) ---------------------------
    for ot in range(4):
        y_sb = ypool.tile([128, HW], F32, name=f"y_sb_{s}_{ot}", tag="y")
        for half in range(2):
            js = [half * 4 + i for i in range(4)]
            pts = hoisted.pop((ot, half), None)
            if pts is None:
                pts = emit_c2x(ot, half)
            for i, j in enumerate(js):
                nc.tensor.matmul(
                    pts[i],
                    lhsT=w2aT[:, ot * 128:(ot + 1) * 128],
                    rhs=out2[:, j * 512:(j + 1) * 512],
                    start=False, stop=True)
            for i, j in enumerate(js):
                # split PSUM->SBUF relu+bias evacuations across DVE and ACT
                if j % 2 == 0:
                    nc.vector.tensor_scalar(
                        out=y_sb[:, j * 512:(j + 1) * 512], in0=pts[i],
                        scalar1=b2[:, ot:ot + 1], scalar2=0.0,
                        op0=mybir.AluOpType.add, op1=mybir.AluOpType.max)
                else:
                    nc.scalar.activation(
                        y_sb[:, j * 512:(j + 1) * 512], pts[i],
                        mybir.ActivationFunctionType.Relu,
                        bias=b2[:, ot:ot + 1], scale=1.0)
            # store each half as soon as its evacuations are done; quarter
            # the very last stores to shrink the kernel tail
            if s == S - 1 and ot == 3:
                for qq in range(2):
                    n0 = half * 2048 + qq * 1024
                    nc.sync.dma_start(out=y_d[s, ot, :, n0:n0 + 1024],
                                      in_=y_sb[:, n0:n0 + 1024])
            else:
                nc.sync.dma_start(
                    out=y_d[s, ot, :, half * 2048:(half + 1) * 2048],
                    in_=y_sb[:, half * 2048:(half + 1) * 2048])


def _build():
    """Build and finalize the per-core Bass program (same on all 8 cores)."""
    nc = bacc.Bacc("TRN2", target_bir_lowering=False, debug=False)

    dram = {
        "x": nc.dram_tensor("x", [S, 128, NCH, HW], BF16, kind="ExternalInput"),
        "att": nc.dram_tensor("att", [S, C8, HW], BF16, kind="ExternalInput"),
        "kwT": nc.dram_tensor("kwT", [128, NCH, C8], BF16, kind="ExternalInput"),
        "kb_bc": nc.dram_tensor("kb_bc", [128, 8, C8], F32, kind="ExternalInput"),
        "w1T": nc.dram_tensor("w1T", [C8, C8], BF16, kind="ExternalInput"),
        "b1": nc.dram_tensor("b1", [C8, 1], F32, kind="ExternalInput"),
        "w2aT": nc.dram_tensor("w2aT", [C8, C], BF16, kind="ExternalInput"),
        "w2bT": nc.dram_tensor("w2bT", [128, NCH, C], BF16, kind="ExternalInput"),
        "b2": nc.dram_tensor("b2", [128, 4], F32, kind="ExternalInput"),
        "y": nc.dram_tensor("y", [S, 4, 128, HW], F32, kind="ExternalOutput"),
    }

    with nc.allow_low_precision("bf16 activations; fp32 accumulate in PSUM"), \
         tile.TileContext(nc) as tc:
        with ExitStack() as ctx:
            consts = ctx.enter_context(tc.tile_pool(name="consts", bufs=1))
            xpool = ctx.enter_context(tc.tile_pool(name="xpool", bufs=2))
            attpool = ctx.enter_context(tc.tile_pool(name="attpool", bufs=2))
            attTpool = ctx.enter_context(tc.tile_pool(name="attTpool", bufs=2))
            projTpool = ctx.enter_context(tc.tile_pool(name="projTpool", bufs=2))
            out2pool = ctx.enter_context(tc.tile_pool(name="out2pool", bufs=2))
            ypool = ctx.enter_context(tc.tile_pool(name="ypool", bufs=2))
            small = ctx.enter_context(tc.tile_pool(name="small", bufs=2))
            psA = ctx.enter_context(tc.tile_pool(name="psA", bufs=2, space="PSUM"))
            psB = ctx.enter_context(tc.tile_pool(name="psB", bufs=2, space="PSUM"))
            psC = ctx.enter_context(tc.tile_pool(name="psC", bufs=4, space="PSUM"))

            # constants: weights into SBUF once. Only what projT needs goes
            # ahead of the first x load; the rest follows.
            sb = {}

            def load_consts(specs):
                for name, shape, dt in specs:
                    t = consts.tile(shape, dt, name=f"{name}_sb")
                    nc.sync.dma_start(out=t, in_=dram[name][:])
                    sb[f"{name}_sb"] = t

            pools = (consts, xpool, attpool, attTpool, projTpool, out2pool,
                     ypool, small, psA, psB, psC)
            dram_all = dict(dram)

            load_consts([("kwT", [128, NCH, C8], BF16),
                         ("kb_bc", [128, 8, C8], F32)])
            loaded = _emit_loads(nc, pools, dram_all, 0, x_first=False)
            load_consts([("w1T", [C8, C8], BF16), ("b1", [C8, 1], F32),
                         ("w2aT", [C8, C], BF16), ("w2bT", [128, NCH, C], BF16),
                         ("b2", [128, 4], F32)])
            ident = consts.tile([C8, C8], BF16, name="ident")
            make_identity(nc, ident)
            sb["ident"] = ident
            expbias = consts.tile([C8, 1], F32, name="expbias")
            nc.vector.memset(expbias, -64.0)
            sb["expbias"] = expbias
            dram_all.update(sb)

            for s in range(S):
                if s + 1 < S:
                    holder = {}

                    def next_loads(s1=s + 1, holder=holder):
                        holder["loaded"] = _emit_loads(nc, pools, dram_all, s1,
                                                       x_first=False)

                    _emit_sample(nc, tc, pools, dram_all, s, loaded, next_loads)
                    loaded = holder["loaded"]
                else:
                    _emit_sample(nc, tc, pools, dram_all, s, loaded, None)

    nc.finalize()
    return nc


def _get_built():
    global _BUILT
    if _BUILT is None:
        _BUILT = _build()
    return _BUILT


def _prep_weights(key_w, key_b, c1_w, c1_b, c1_gamma, c1_beta, c1_mean, c1_var,
                  c2_w, c2_b, c2_gamma, c2_beta, c2_mean, c2_var):
    s1 = c1_gamma / np.sqrt(c1_var + EPS)
    w1 = c1_w * s1[:, None]                       # (64, 64)
    b1 = c1_b * s1 + c1_beta - c1_mean * s1       # (64,)
    s2 = c2_gamma / np.sqrt(c2_var + EPS)
    w2 = c2_w * s2[:, None]                       # (512, 576)
    b2 = c2_b * s2 + c2_beta - c2_mean * s2       # (512,)
    w2a = w2[:, :C8]                              # (512, 64)  applies to out2
    w2b = w2[:, C8:]                              # (512, 512) applies to x

    return {
        "kwT": np.ascontiguousarray(
            key_w.T.reshape(NCH, 128, C8).transpose(1, 0, 2)).astype(NP_BF16),
        "kb_bc": np.ascontiguousarray(
            np.broadcast_to(key_b[None, None, :], (128, 8, C8))).astype(np.float32),
        "w1T": np.ascontiguousarray(w1.T).astype(NP_BF16),
        "b1": b1.reshape(C8, 1).astype(np.float32),
        "w2aT": np.ascontiguousarray(w2a.T).astype(NP_BF16),
        "w2bT": np.ascontiguousarray(
            w2b.T.reshape(NCH, 128, C).transpose(1, 0, 2)).astype(NP_BF16),
        "b2": np.ascontiguousarray(b2.reshape(4, 128).T).astype(np.float32),
    }


def kernel(**inputs):
    nc = _get_built()

    x = np.asarray(inputs["x"], np.float32).reshape(B, C, HW)
    att = np.asarray(inputs["att"], np.float32).reshape(B, C8, HW)
    weights = _prep_weights(**{k: np.asarray(v, np.float32)
                               for k, v in inputs.items()
                               if k not in ("x", "att")})

    in_maps = []
    for c in range(N_CORES):
        s0 = c * S
        x_core = np.ascontiguousarray(
            x[s0:s0 + S].reshape(S, NCH, 128, HW).transpose(0, 2, 1, 3)
        ).astype(NP_BF16)
        att_core = att[s0:s0 + S].astype(NP_BF16)
        m = {"x": x_core, "att": att_core}
        m.update(weights)
        in_maps.append(m)

    res = run_bass_kernel_spmd(nc, in_maps, core_ids=list(range(N_CORES)))

    y = np.concatenate([res.results[c]["y"] for c in range(N_CORES)], axis=0)
    return np.ascontiguousarray(y.reshape(B, C, H, W)).astype(np.float32)

